# revision 23
# baseline (speedup 1.0000x reference)
"""Trainium2 Bass kernel for windowed Conv1d(k=3) + sigmoid gating.

Reference computation (B=16, T=960, D=1024, W=10):
  windows of size 10 are conv'd independently with per-window zero pad 1:
    cnn[t, d] = sum_{k,c} conv_w[d, c, k] * xpad[t + k, c] + conv_b[d]
    out = cnn * sigmoid(cnn @ gate_w.T + gate_b)

Strategy: data parallelism over the 8 NeuronCores (2 batches / 192
windows per core).  The conv runs as mixed Winograd: each window's 10
outputs come from two F(4,3) tiles (xpad[0:6] -> y0:4, xpad[4:10] ->
y4:8) plus one F(2,3) tile (xpad[8:12] -> y8:10) - 16 matmul streams
per window instead of plain F(2,3)'s 20.  All matmuls are bf16 with
f32 PSUM accumulation (fp8 DoubleRow measures at the same one-output-
column-per-cycle rate as bf16 on this hardware, so it buys nothing).

Hardware constraints that shape the schedule:
- A matmul's ~97ns LDWEIGHTS only hides under the previous matmul's
  stream, so the moving width must be >=~240 columns: conv streams span
  ALL FOUR column groups (F(4,3): 384 wide, F(2,3): 192).
- The 10 m-streams x 8 output chunks are walked STREAM-ROW-MAJOR
  (p outer, dck inner): row p's input strip and weight strip feed all
  8 dck matmul groups, so the DMA queue (xt strip 0.8MB + weight strip
  2.1MB per row) paces just ahead of the PE (~10us per row) from the
  very first chunk - no multi-MB prefetch bubble.  m-terms are staged
  to SBUF bf16 as each stream closes, so only ~3 single-bank PSUM slots
  are ever live, and the A^T combine work is spread across later rows.
- The 21MB of winograd weights stream through a 2-deep strip rotation
  (16KB/partition each); input strips rotate 3-deep.

The gate matmul is bf16, runs after all convs with the four groups'
ecks interleaved through a 4-slot PSUM rotation.  Output is stored
bf16 and widened to f32 on the host.
"""

import numpy as np
import ml_dtypes

import concourse.bacc as bacc
import concourse.bass as bass
import concourse.tile as tile
from concourse import mybir
from concourse.bass_utils import run_bass_kernel_spmd

BF16 = ml_dtypes.bfloat16

B, T, D, W = 16, 960, 1024, 10
NCORES = 8
BC = B // NCORES            # batches per core
NWIN = BC * T // W          # windows per core (192)
RC = NWIN * W               # output rows per core (1920)
PW = W + 2                  # padded window length (12)
NG = 4                      # column groups per core
GWIN = NWIN // NG           # windows per group (48)
GN = GWIN * W               # output columns per group (480)
NCH = D // 128              # 128-partition chunks of D (8)
N43 = NG * 2 * GWIN         # F(4,3) stream width (384)
N23 = NG * GWIN             # F(2,3) stream width (192)
XB43 = NCH * N43            # xt strip cols per F43 stream (3072)
XB23 = NCH * N23            # xt strip cols per F23 stream (1536)
WSTRIP = NCH * NCH * 128    # weight strip cols per stream row (8192)
AF = mybir.ActivationFunctionType

# stream processing order: F43 m-index [1,2,3,4,0,5], F23 m-index [1,2,0,3]
P43 = [1, 2, 3, 4, 0, 5]
P23 = [1, 2, 0, 3]


def _pw(p):
    return N43 if p < 6 else N23


def _build():
    nc = bacc.Bacc("TRN2", target_bir_lowering=False, debug=False)

    # xtA[p]: winograd-transformed input strip for stream row p:
    # [cc, (ck, g, tile, win)] (F23 rows have half-width cols, zero-padded
    # dram rows for uniform shape)
    xtA = nc.dram_tensor("xtA", [10, 128, XB43], mybir.dt.bfloat16,
                         kind="ExternalInput")
    # cwrP[p]: weight strip [cc, (dck*8+ck)*128+dd] = WtP[p][dck*128+dd, ...]
    cwrP = nc.dram_tensor("cwrP", [10, 128, WSTRIP], mybir.dt.bfloat16,
                          kind="ExternalInput")
    gwr = nc.dram_tensor("gwr", [NCH, 128, NCH * 128], mybir.dt.bfloat16,
                         kind="ExternalInput")
    cb = nc.dram_tensor("cb", [128, NCH], mybir.dt.float32, kind="ExternalInput")
    gb = nc.dram_tensor("gb", [128, NCH], mybir.dt.float32, kind="ExternalInput")
    outT = nc.dram_tensor("outT", [D, RC], mybir.dt.bfloat16,
                          kind="ExternalOutput")

    with tile.TileContext(nc) as tc:
        with (
            tc.tile_pool(name="consts", bufs=1) as consts,
            tc.tile_pool(name="work", bufs=2) as work,
            tc.tile_pool(name="xtp", bufs=3) as xtp,
            tc.tile_pool(name="cwp", bufs=2) as cwp,
            tc.tile_pool(name="cpsum", bufs=3, space="PSUM") as cpsum,
            tc.tile_pool(name="gpsum", bufs=1, space="PSUM") as gpsum,
        ):
            cb_first = [True]

            def load_strip(p):
                # each dma_start costs ~600ns of SP trigger processing, so
                # strips move as whole transfers; only row 0 is split in two
                # so its first matmuls start ~4us in.
                w = _pw(p)
                xts = xtp.tile([128, XB43], mybir.dt.bfloat16, tag="xts",
                               name=f"xts{p}")
                wst = cwp.tile([128, WSTRIP], mybir.dt.bfloat16, tag="wst",
                               name=f"wst{p}")
                nch = NCH // 2 if p == 0 else NCH
                for lo in range(0, NCH, nch):
                    hi = lo + nch
                    nc.sync.dma_start(xts[:, lo * w:hi * w],
                                      xtA[p][:, lo * w:hi * w])
                    nc.sync.dma_start(
                        wst[:, lo * NCH * 128:hi * NCH * 128],
                        cwrP[p][:, lo * NCH * 128:hi * NCH * 128])
                if cb_first[0]:
                    cb_first[0] = False
                    nc.sync.dma_start(cb_sb[:], cb[:])
                    nc.sync.dma_start(gb_sb[:], gb[:])
                return xts, wst

            cb_sb = consts.tile([128, NCH], mybir.dt.float32, tag="cb")
            gb_sb = consts.tile([128, NCH], mybir.dt.float32, tag="gb")

            cnnAll = [
                consts.tile([128, NG * GN], mybir.dt.bfloat16,
                            tag=f"cnnAll{d}", name=f"cnnAll{d}")
                for d in range(NCH)
            ]
            # staged m-terms / combine subexpressions, one slice per dck
            def allt(name, wid=N43):
                return consts.tile([128, NCH * wid], mybir.dt.bfloat16,
                                   tag=name, name=name)

            m1sA = allt("m1sA")
            m3sA = allt("m3sA")
            m0sA = allt("m0sA")
            m5sA = allt("m5sA")
            s12A = allt("s12A")
            d12A = allt("d12A")
            s34A = allt("s34A")
            d34A = allt("d34A")
            n1sA = allt("n1sA", N23)
            n2sA = allt("n2sA", N23)

            scr = consts.tile([128, 512], mybir.dt.bfloat16, tag="scr")
            nc.gpsimd.memset(scr[:], 0.0)
            wps = gpsum.tile([128, 512], mybir.dt.float32, tag="gpsA")
            for _ in range(14):
                nc.tensor.matmul(wps[:, :480], scr[:, :128], scr[:, :480],
                                 start=True, stop=True)

            strips = {}
            strips[0] = load_strip(0)
            strips[1] = load_strip(1)

            def mm(p, dck):
                w = _pw(p)
                xts, wst = strips[p]
                ps = cpsum.tile([128, 512], mybir.dt.float32, tag="cps",
                                name=f"cps{p}_{dck}")
                for ck in range(NCH):
                    nc.tensor.matmul(
                        ps[:, :w],
                        wst[:, (dck * NCH + ck) * 128:
                            (dck * NCH + ck + 1) * 128],
                        xts[:, ck * w:(ck + 1) * w],
                        start=(ck == 0),
                        stop=(ck == NCH - 1),
                    )
                return ps

            def tmp(wid=N43):
                return work.tile([128, wid], mybir.dt.bfloat16, tag="tmp",
                                 bufs=6, name="tmp")

            def sl(t, dck, wid=N43):
                return t[:, dck * wid:(dck + 1) * wid]

            def v43(ap):
                return ap.rearrange("q (g u w) -> q g u w", g=NG, u=2)

            def y43(dck, i, in1, in2):
                ct = cnnAll[dck][:].rearrange("q (g t w) -> q g t w",
                                              g=NG, w=GWIN)
                i1, i2 = v43(in1), v43(in2)
                for u in (0, 1):
                    nc.vector.tensor_add(ct[:, :, u * 4 + i, :],
                                         i1[:, :, u, :], i2[:, :, u, :])

            # y-op batches, spread across rows to balance the vector load
            def batch_y0(dck):
                cbs = cb_sb[:, dck:dck + 1]
                t1 = tmp()
                nc.vector.tensor_scalar_add(t1[:], sl(m0sA, dck), cbs)
                t2 = tmp()
                nc.vector.tensor_add(t2[:], t1[:], sl(s12A, dck))
                y43(dck, 0, t2[:], sl(s34A, dck))      # y0 = m0+s12+s34+cb

            def batch_y1(dck):
                cbs = cb_sb[:, dck:dck + 1]
                e2 = tmp()
                nc.vector.tensor_scalar_mul(e2[:], sl(d34A, dck), 2.0)
                t3 = tmp()
                nc.vector.tensor_scalar_add(t3[:], sl(d12A, dck), cbs)
                y43(dck, 1, t3[:], e2[:])              # y1 = d12+2*d34+cb

            def batch_y2(dck):
                cbs = cb_sb[:, dck:dck + 1]
                f4 = tmp()
                nc.vector.tensor_scalar_mul(f4[:], sl(s34A, dck), 4.0)
                t4 = tmp()
                nc.vector.tensor_scalar_add(t4[:], f4[:], cbs)
                y43(dck, 2, t4[:], sl(s12A, dck))      # y2 = s12+4*s34+cb

            def batch_y3(dck):
                cbs = cb_sb[:, dck:dck + 1]
                e8 = tmp()
                nc.vector.tensor_scalar_mul(e8[:], sl(d34A, dck), 8.0)
                t5 = tmp()
                nc.vector.tensor_scalar_add(t5[:], sl(m5sA, dck), cbs)
                t6 = tmp()
                nc.vector.tensor_add(t6[:], t5[:], e8[:])
                y43(dck, 3, t6[:], sl(d12A, dck))      # y3 = d12+8*d34+m5+cb

            def row_post(p, dck, ps):
                """per-stream epilogue: stage / combine for this row"""
                if p == 0:      # F43 m1
                    nc.scalar.activation(sl(m1sA, dck), ps[:, :N43], AF.Copy)
                elif p == 1:    # F43 m2
                    nc.vector.tensor_add(sl(s12A, dck), ps[:, :N43],
                                         sl(m1sA, dck))
                    nc.vector.tensor_sub(sl(d12A, dck), sl(m1sA, dck),
                                         ps[:, :N43])
                elif p == 2:    # F43 m3
                    nc.scalar.activation(sl(m3sA, dck), ps[:, :N43], AF.Copy)
                elif p == 3:    # F43 m4
                    nc.vector.tensor_add(sl(s34A, dck), ps[:, :N43],
                                         sl(m3sA, dck))
                    nc.vector.tensor_sub(sl(d34A, dck), sl(m3sA, dck),
                                         ps[:, :N43])
                elif p == 4:    # F43 m0
                    nc.scalar.activation(sl(m0sA, dck), ps[:, :N43], AF.Copy)
                elif p == 5:    # F43 m5
                    nc.scalar.activation(sl(m5sA, dck), ps[:, :N43], AF.Copy)
                    batch_y0(dck)
                elif p == 6:    # F23 m1
                    nc.scalar.activation(sl(n1sA, dck, N23), ps[:, :N23],
                                         AF.Copy)
                    batch_y1(dck)
                elif p == 7:    # F23 m2
                    nc.scalar.activation(sl(n2sA, dck, N23), ps[:, :N23],
                                         AF.Copy)
                    batch_y2(dck)
                elif p == 8:    # F23 m0
                    cbs = cb_sb[:, dck:dck + 1]
                    ct = cnnAll[dck][:].rearrange("q (g t w) -> q g t w",
                                                  g=NG, w=GWIN)
                    t0 = tmp(N23)
                    nc.vector.tensor_scalar_add(t0[:], ps[:, :N23], cbs)
                    a2 = tmp(N23)
                    nc.vector.tensor_add(a2[:], t0[:], sl(n1sA, dck, N23))
                    nc.vector.tensor_add(ct[:, :, 8, :], a2[:],
                                         sl(n2sA, dck, N23))
                    batch_y3(dck)
                else:           # p == 9, F23 m3
                    cbs = cb_sb[:, dck:dck + 1]
                    ct = cnnAll[dck][:].rearrange("q (g t w) -> q g t w",
                                                  g=NG, w=GWIN)
                    u = tmp(N23)
                    nc.vector.tensor_scalar(u[:], ps[:, :N23], cbs, None,
                                            mybir.AluOpType.subtract)
                    v = tmp(N23)
                    nc.vector.tensor_sub(v[:], sl(n1sA, dck, N23),
                                         sl(n2sA, dck, N23))
                    nc.vector.tensor_sub(ct[:, :, 9, :], v[:], u[:])

            for p in range(10):
                for dck in range(NCH):
                    ps = mm(p, dck)
                    row_post(p, dck, ps)
                if p + 2 < 10:
                    strips[p + 2] = load_strip(p + 2)
                if p == 7:
                    gwr_sb = []
                    for eck in range(NCH):
                        t = consts.tile([128, NCH * 128], mybir.dt.bfloat16,
                                        tag=f"gw{eck}", name=f"gw{eck}")
                        nc.sync.dma_start(t[:], gwr[eck])
                        gwr_sb.append(t)

            gtag = [0]

            def gate_eck(g, eck, last=False):
                # 7-deep psum rotation: the 4 gate banks plus the 3 conv
                # slots (idle once the conv phase ends)
                k = gtag[0] % 7
                gtag[0] += 1
                if k < 4:
                    ps2 = gpsum.tile([128, 512], mybir.dt.float32,
                                     tag=f"gps{'ABCD'[k]}",
                                     name=f"ps2{'ABCD'[k]}")
                else:
                    ps2 = cpsum.tile([128, 512], mybir.dt.float32, tag="cps",
                                     name=f"ps2c{k}")
                for dck in range(NCH):
                    nc.tensor.matmul(
                        ps2[:, :GN],
                        gwr_sb[eck][:, dck * 128:(dck + 1) * 128],
                        cnnAll[dck][:, g * GN:(g + 1) * GN],
                        start=(dck == 0),
                        stop=(dck == NCH - 1),
                    )
                gt = work.tile([128, GN], mybir.dt.bfloat16, tag="gate", bufs=4)
                ot = work.tile([128, GN], mybir.dt.bfloat16, tag="out", bufs=4)
                # half-tile every epilogue so sigmoid/mul/store pipeline
                # against the matmul stream instead of serializing per eck
                for lo, hi in ((0, GN // 2), (GN // 2, GN)):
                    nc.scalar.activation(gt[:, lo:hi], ps2[:, lo:hi],
                                         AF.Sigmoid,
                                         bias=gb_sb[:, eck:eck + 1])
                    nc.vector.tensor_mul(ot[:, lo:hi],
                                         cnnAll[eck][:, g * GN + lo:g * GN + hi],
                                         gt[:, lo:hi])
                    nc.sync.dma_start(
                        outT[eck * 128:(eck + 1) * 128,
                             g * GN + lo:g * GN + hi], ot[:, lo:hi]
                    )

            for eck in range(NCH):
                for g in range(NG):
                    gate_eck(g, eck, last=(eck == NCH - 1 and g == NG - 1))
    nc.compile()
    return nc


# Winograd transforms (Lavin): F(4,3) with points {0,+-1,+-2,inf}
G4 = np.array([
    [1 / 4, 0, 0],
    [-1 / 6, -1 / 6, -1 / 6],
    [-1 / 6, 1 / 6, -1 / 6],
    [1 / 24, 1 / 12, 1 / 6],
    [1 / 24, -1 / 12, 1 / 6],
    [0, 0, 1]], np.float32)
BT4 = np.array([
    [4, 0, -5, 0, 1, 0],
    [0, -4, -4, 1, 1, 0],
    [0, 4, -4, -1, 1, 0],
    [0, -2, -1, 2, 1, 0],
    [0, 2, -1, -2, 1, 0],
    [0, 4, 0, -5, 0, 1]], np.float32)
G2 = np.array([[1, 0, 0], [.5, .5, .5], [.5, -.5, .5], [0, 0, 1]], np.float32)
BT2 = np.array([
    [1, 0, -1, 0], [0, 1, 1, 0], [0, -1, 1, 0], [0, 1, 0, -1]], np.float32)


def _prep_core_input(x_shard, cw_host, gw_host, cb_host, gb_host):
    # x_shard: [BC, T, D] -> padded transposed xp [D, NG, PW, GWIN]
    xs = x_shard.reshape(NG, GWIN, W, D)
    xp = np.zeros((D, NG, PW, GWIN), np.float32)
    xp[:, :, 1:1 + W, :] = xs.transpose(3, 0, 2, 1)
    # F43 tiles: xpad[0:6] and xpad[4:10]
    d43 = np.stack([xp[:, :, 0:6, :], xp[:, :, 4:10, :]], axis=2)  # [D,NG,2,6,GW]
    xt43 = np.einsum('jk,dgukw->jdguw', BT4, d43)   # [6, D, NG, 2, GWIN]
    xt23 = np.einsum('jk,dgkw->jdgw', BT2, xp[:, :, 8:12, :])  # [4, D, NG, GWIN]
    xt_host = np.zeros((10, 128, XB43), np.float32)
    b43 = xt43[P43].reshape(6, NCH, 128, N43).transpose(0, 2, 1, 3)
    xt_host[:6] = b43.reshape(6, 128, XB43)
    b23 = xt23[P23].reshape(4, NCH, 128, N23).transpose(0, 2, 1, 3)
    xt_host[6:, :, :XB23] = b23.reshape(4, 128, XB23)
    return {"xtA": xt_host.astype(BF16), "cwrP": cw_host, "gwr": gw_host,
            "cb": cb_host, "gb": gb_host}


def _prep_in_maps(x, conv_w, conv_b, gate_w, gate_b):
    wk = conv_w.transpose(2, 0, 1)                  # [k, d, c]
    wt43 = np.einsum('jk,kdc->jdc', G4, wk)         # [6, D, D]
    wt23 = np.einsum('jk,kdc->jdc', G2, wk)         # [4, D, D]
    wtP = np.stack(list(wt43[P43]) + list(wt23[P23]))  # [10, D, D]
    # cwrP[p][cc, (dck*8+ck)*128+dd] = wtP[p][dck*128+dd, ck*128+cc]
    arr = wtP.reshape(10, NCH, 128, NCH, 128)       # [p, dck, dd, ck, cc]
    cw_host = np.ascontiguousarray(arr.transpose(0, 4, 1, 3, 2)).reshape(
        10, 128, WSTRIP).astype(BF16)
    # gate lhsT blocks: gwr[eck][dd, dck*128 + ee] = gate_w[eck*128+ee, dck*128+dd]
    gwt = gate_w.T.reshape(NCH, 128, NCH, 128)      # [dck, dd, eck, ee]
    gw_host = np.ascontiguousarray(gwt.transpose(2, 1, 0, 3)).reshape(
        NCH, 128, NCH * 128).astype(BF16)
    cb_host = np.ascontiguousarray(conv_b.reshape(NCH, 128).T).astype(np.float32)
    gb_host = np.ascontiguousarray(gate_b.reshape(NCH, 128).T).astype(np.float32)
    return [
        _prep_core_input(x[BC * i:BC * (i + 1)], cw_host, gw_host, cb_host,
                         gb_host)
        for i in range(NCORES)
    ]


def _unshard_core(o):
    # o: [D, RC] with columns ordered (group, t, win) -> [BC, T, D]
    return (np.asarray(o).astype(np.float32)
            .reshape(D, NG, W, GWIN).transpose(1, 3, 2, 0)
            .reshape(NWIN, W, D).reshape(BC, T, D))


_NC_CACHE = None


def kernel(x, conv_w, conv_b, gate_w, gate_b):
    global _NC_CACHE
    x = np.asarray(x, np.float32)
    conv_w = np.asarray(conv_w, np.float32)
    conv_b = np.asarray(conv_b, np.float32)
    gate_w = np.asarray(gate_w, np.float32)
    gate_b = np.asarray(gate_b, np.float32)

    in_maps = _prep_in_maps(x, conv_w, conv_b, gate_w, gate_b)
    if _NC_CACHE is None:
        _NC_CACHE = _build()
    res = run_bass_kernel_spmd(_NC_CACHE, in_maps, core_ids=list(range(NCORES))).results

    out = np.empty((B, T, D), np.float32)
    for i in range(NCORES):
        out[BC * i:BC * (i + 1)] = _unshard_core(res[i]["outT"])
    return out
